# revision 41
# baseline (speedup 1.0000x reference)
"""Trainium2 Bass kernel for LinearMemoryAttention (B=1, S=4096, D=512, H=8, Dh=64).

v3: sequence-parallel over 8 cores (512 tokens each), all heads local.
- bf16 matmul operands throughout (fp32 PSUM accumulation).
- Projections computed feature-major so biases fuse into activations.
- Cross-core causal state exchanged through shared-HBM scratchpad: each
  core scatters its block-sum into its rank's slot (indirect DMA, slot
  index supplied as a per-core input), announces completion with a
  remote semaphore broadcast (SWDGE, no ncfw collective), then gathers
  all 8 slots with one DMA. A 1-byte prelude kernel barrier provides
  entry sync across invocations.

Self-contained: hardcodes all shapes; builds/compiles the Bass program once.
"""

import os

import numpy as np

import concourse.bass as bass
import concourse.bacc as bacc
import concourse.mybir as mybir
import concourse.tile as tile
from concourse.bass_utils import run_bass_kernel_spmd

F32 = mybir.dt.float32
BF16 = mybir.dt.bfloat16
U32 = mybir.dt.uint32
N_CORES = 8
S = 4096
D = 512
H = 8
DH = 64
HP = 66  # head width incl. denominator column (+1 pad)
S_BLK = S // N_CORES  # 512 rows per core
NCH = S_BLK // 128  # 4 chunks of 128
NHP = H // 2  # 4 head pairs
EPS = 1e-6
W = NHP * HP  # 264
AUXW = 4 + 4 + 4 + W + N_CORES  # bqt | bkt | bvt | mz | pmask

_CACHE = {}
DEBUG = os.environ.get("LMA_DEBUG", "")  # "" or "noremote"


def _build():
    Alu = mybir.AluOpType
    Act = mybir.ActivationFunctionType
    nc = bacc.Bacc("TRN2", target_bir_lowering=False, debug=False,
                   num_devices=N_CORES)

    hs_d = nc.dram_tensor("hs", [S_BLK, D], F32, kind="ExternalInput").ap()
    wq_d = nc.dram_tensor("wq", [D, D], F32, kind="ExternalInput").ap()
    wk_d = nc.dram_tensor("wk", [D, D], F32, kind="ExternalInput").ap()
    wv_d = nc.dram_tensor("wv", [D, D], F32, kind="ExternalInput").ap()
    wo_d = nc.dram_tensor("wo", [D, D], F32, kind="ExternalInput").ap()
    aux_d = nc.dram_tensor("aux", [128, AUXW], F32, kind="ExternalInput").ap()
    sidx_d = nc.dram_tensor("sidx", [128, 1], U32, kind="ExternalInput").ap()
    y_d = nc.dram_tensor("y", [S_BLK, D], F32, kind="ExternalOutput").ap()

    rsem = nc.alloc_semaphore("lma_rsem")
    lsem = nc.alloc_semaphore("lma_lsem")
    dsem = nc.alloc_semaphore("lma_dsem")

    with tile.TileContext(nc) as tc:
        with (
            tc.tile_pool(name="const", bufs=1) as cpool,
            tc.tile_pool(name="wstage", bufs=1) as wspool,
            tc.tile_pool(name="wpool", bufs=1) as wpool,
            tc.tile_pool(name="data", bufs=1) as dpool,
            tc.tile_pool(name="tmp", bufs=3) as tpool,
            tc.tile_pool(name="small", bufs=4) as spool,
            tc.tile_pool(name="dram", bufs=1, space="DRAM") as drpool,
        ):
            # ---- input DMAs (one issue per tensor, sync queue = idle) ------
            hs_t = dpool.tile([128, NCH * D], F32, name="hsall")
            nc.sync.dma_start(
                hs_t.rearrange("p (c d) -> p c d", c=NCH),
                hs_d.rearrange("(c p) d -> p c d", p=128))
            aux = cpool.tile([128, AUXW], F32)
            nc.sync.dma_start(aux[:], aux_d[:])
            sidx = cpool.tile([128, 1], U32)
            nc.sync.dma_start(sidx[:], sidx_d[:])
            wk_s = wspool.tile([128, 4 * D], F32, name="wks")
            nc.sync.dma_start(
                wk_s.rearrange("p (c d) -> p c d", c=4),
                wk_d.rearrange("(c p) d -> p c d", p=128))
            wv_s = wspool.tile([128, 4 * D], F32, name="wvs")
            nc.sync.dma_start(
                wv_s.rearrange("p (c d) -> p c d", c=4),
                wv_d.rearrange("(c p) d -> p c d", p=128))
            wq_s = wspool.tile([128, 4 * D], F32, name="wqs")
            nc.sync.dma_start(
                wq_s.rearrange("p (c d) -> p c d", c=4),
                wq_d.rearrange("(c p) d -> p c d", p=128))
            wo_s = wspool.tile([128, 4 * D], F32, name="wos")
            nc.sync.dma_start(
                wo_s.rearrange("p (c d) -> p c d", c=4),
                wo_d.rearrange("(c p) d -> p c d", p=128))

            bqt = aux[:, 0:4]
            bkt = aux[:, 4:8]
            bvt = aux[:, 8:12]
            mz = aux[:, 12:12 + W]
            pmask = aux[:, 12 + W:12 + W + N_CORES]

            # ---- constants -------------------------------------------------
            ones128 = cpool.tile([128, 128], BF16)
            nc.gpsimd.memset(ones128[:], 1.0)
            ident = cpool.tile([128, 128], BF16)
            nc.gpsimd.affine_select(ident[:], ones128[:], pattern=[[1, 128]],
                                    compare_op=Alu.is_equal, fill=0.0, base=0,
                                    channel_multiplier=-1)
            triu = cpool.tile([128, 128], BF16)
            nc.gpsimd.affine_select(triu[:], ones128[:], pattern=[[1, 128]],
                                    compare_op=Alu.is_ge, fill=0.0, base=0,
                                    channel_multiplier=-1)

            # ---- cast hs + weights to bf16 --------------------------------
            hs_b = dpool.tile([128, NCH * D], BF16, name="hsb")
            for c in range(NCH):
                nc.vector.tensor_copy(hs_b[:, c * D:(c + 1) * D],
                                      hs_t[:, c * D:(c + 1) * D])
            wk_t = [wpool.tile([128, D], BF16, name=f"wk{i}") for i in range(4)]
            wv_t = [wpool.tile([128, D], BF16, name=f"wv{i}") for i in range(4)]
            wq_t = [wpool.tile([128, D], BF16, name=f"wq{i}") for i in range(4)]
            wo_t = [wpool.tile([128, D], BF16, name=f"wo{i}") for i in range(4)]
            for i in range(4):
                sl = slice(i * D, (i + 1) * D)
                nc.vector.tensor_copy(wk_t[i][:], wk_s[:, sl])
                nc.vector.tensor_copy(wv_t[i][:], wv_s[:, sl])

            # ---- cross-core exchange buffers ------------------------------
            ball = dpool.tile([128, W], BF16, name="ball")
            slots = dpool.tile([128, N_CORES * W], BF16, name="slots")


            hsT = [dpool.tile([128, S_BLK], BF16, name=f"hsT{i}")
                   for i in range(4)]
            skT = [dpool.tile([128, S_BLK], BF16, name=f"skT{hp}")
                   for hp in range(NHP)]
            sqT = [dpool.tile([128, S_BLK], BF16, name=f"sqT{hp}")
                   for hp in range(NHP)]
            vT = [dpool.tile([128, S_BLK], BF16, name=f"vT{hp}")
                  for hp in range(NHP)]
            sk_tm = [dpool.tile([128, H * DH], BF16, name=f"sk{c}")
                     for c in range(NCH)]
            v_tm = [dpool.tile([128, H * HP], BF16, name=f"v{c}")
                    for c in range(NCH)]
            for c in range(NCH):
                v3 = v_tm[c].rearrange("p (h e) -> p h e", e=HP)
                nc.gpsimd.memset(v3[:, :, DH:HP], 1.0)
            L_sb = [None] + [dpool.tile([128, W], F32, name=f"L{c}")
                             for c in range(1, NCH)]
            ball_f = dpool.tile([128, W], F32, name="ballf")

            with tc.tile_pool(name="ps", bufs=1, space="PSUM") as ps:
                # ---- hs transposes -----------------------------------------
                for dt in range(4):
                    for st in range(4):
                        pst = ps.tile([128, 128], BF16, name="pstr", bufs=2)
                        nc.tensor.transpose(
                            pst[:],
                            hs_b[:, st * D + dt * 128: st * D + (dt + 1) * 128],
                            ident[:])
                        nc.vector.tensor_copy(
                            hsT[dt][:, st * 128:(st + 1) * 128], pst[:])

                # ---- k, v projections (feature-major), elu(k)+1 ------------
                for hp in range(NHP):
                    fs = slice(hp * 128, (hp + 1) * 128)
                    psk = ps.tile([128, S_BLK], F32, name="psbig", bufs=2)
                    for dt in range(4):
                        nc.tensor.matmul(psk[:], wk_t[dt][:, fs], hsT[dt][:],
                                         start=(dt == 0), stop=(dt == 3))
                    e_t = tpool.tile([128, S_BLK], BF16, name="elu_e")
                    r_t = tpool.tile([128, S_BLK], BF16, name="elu_r")
                    nc.scalar.activation(e_t[:], psk[:], Act.Exp,
                                         bias=bkt[:, hp:hp + 1])
                    nc.vector.tensor_scalar(r_t[:], psk[:], bkt[:, hp:hp + 1],
                                            0.0, op0=Alu.add, op1=Alu.max)
                    nc.vector.scalar_tensor_tensor(
                        skT[hp][:], e_t[:], 1.0, r_t[:],
                        op0=Alu.min, op1=Alu.add)

                    psv = ps.tile([128, S_BLK], F32, name="psbig", bufs=2)
                    for dt in range(4):
                        nc.tensor.matmul(psv[:], wv_t[dt][:, fs], hsT[dt][:],
                                         start=(dt == 0), stop=(dt == 3))
                    nc.scalar.activation(vT[hp][:], psv[:], Act.Identity,
                                         bias=bvt[:, hp:hp + 1])

                # ---- transpose sk, v to token-major ------------------------
                for c in range(NCH):
                    cs = slice(c * 128, (c + 1) * 128)
                    for hp in range(NHP):
                        pst = ps.tile([128, 128], BF16, name="pstr", bufs=2)
                        nc.tensor.transpose(pst[:], skT[hp][:, cs], ident[:])
                        nc.vector.tensor_copy(
                            sk_tm[c][:, hp * 128:(hp + 1) * 128], pst[:])
                        pst2 = ps.tile([128, 128], BF16, name="pstr", bufs=2)
                        nc.tensor.transpose(pst2[:], vT[hp][:, cs], ident[:])
                        v3 = v_tm[c].rearrange("p (h e) -> p h e", e=HP)
                        nc.scalar.copy(
                            v3[:, 2 * hp:2 * hp + 2, 0:DH],
                            pst2.rearrange("p (h e) -> p h e", e=DH))

                # ---- U outer products -> local prefix + block total --------
                for st in range(NCH):
                    for hp in range(NHP):
                        psU = ps.tile([128, HP], F32, name="psu", bufs=2,
                                      padded_shape=[128, 512])
                        for sub in range(2):
                            h = 2 * hp + sub
                            nc.tensor.matmul(
                                psU[sub * 64:(sub + 1) * 64, :],
                                sk_tm[st][:, h * DH:(h + 1) * DH],
                                v_tm[st][:, h * HP:(h + 1) * HP],
                                start=True, stop=True,
                                tile_position=(0, 64 * sub))
                        dest = L_sb[st + 1] if st < NCH - 1 else ball_f
                        dsl = slice(hp * HP, (hp + 1) * HP)
                        nc.scalar.copy(dest[:, dsl], psU[:])

                for st in range(2, NCH):
                    nc.gpsimd.tensor_add(L_sb[st][:], L_sb[st][:],
                                         L_sb[st - 1][:])
                nc.gpsimd.tensor_add(ball_f[:], ball_f[:],
                                     L_sb[NCH - 1][:])
                nc.scalar.copy(ball[:], ball_f[:])

                # ---- exchange: AllGather block totals (bf16 payload) -------
                if DEBUG != "noremote":
                    cc_in = drpool.tile([128, W], BF16, name="ccin")
                    cc_out = drpool.tile([N_CORES, 128, W], BF16,
                                         addr_space="Shared", name="ccout")
                    nc.sync.dma_start(cc_in[:], ball[:])
                    nc.gpsimd.collective_compute(
                        "AllGather", Alu.bypass,
                        replica_groups=[list(range(N_CORES))],
                        ins=[cc_in[:]], outs=[cc_out[:]])

                # ---- deferred weight casts (off the U critical path) -------
                for i in range(4):
                    sl = slice(i * D, (i + 1) * D)
                    nc.scalar.copy(wq_t[i][:], wq_s[:, sl])
                    nc.scalar.copy(wo_t[i][:], wo_s[:, sl])

                # ---- q projections (overlap the exchange) ------------------
                for hp in range(NHP):
                    fs = slice(hp * 128, (hp + 1) * 128)
                    psq = ps.tile([128, S_BLK], F32, name="psbig", bufs=2)
                    for dt in range(4):
                        nc.tensor.matmul(psq[:], wq_t[dt][:, fs], hsT[dt][:],
                                         start=(dt == 0), stop=(dt == 3))
                    e_t = tpool.tile([128, S_BLK], BF16, name="elu_e")
                    r_t = tpool.tile([128, S_BLK], BF16, name="elu_r")
                    nc.scalar.activation(e_t[:], psq[:], Act.Exp,
                                         bias=bqt[:, hp:hp + 1])
                    nc.vector.tensor_scalar(r_t[:], psq[:], bqt[:, hp:hp + 1],
                                            0.0, op0=Alu.add, op1=Alu.max)
                    nc.vector.scalar_tensor_tensor(
                        sqT[hp][:], e_t[:], 1.0, r_t[:],
                        op0=Alu.min, op1=Alu.add)

                # ---- masked scores + intra-chunk numerator (overlap cc) ----
                ni_sb = [[None] * NHP for _ in range(NCH)]
                for c in range(NCH):
                    cs = slice(c * 128, (c + 1) * 128)
                    for hp in range(NHP):
                        am = dpool.tile([128, 256], BF16, name=f"am{c}_{hp}")
                        ni = dpool.tile([128, 2 * HP], F32, name=f"ni{c}_{hp}")
                        for sub in range(2):
                            h = 2 * hp + sub
                            hb = slice(sub * 64, (sub + 1) * 64)
                            psA = ps.tile([128, 128], F32, name="psa", bufs=2)
                            nc.tensor.matmul(psA[:], skT[hp][hb, cs],
                                             sqT[hp][hb, cs],
                                             start=True, stop=True)
                            nc.vector.tensor_mul(
                                am[:, sub * 128:(sub + 1) * 128],
                                psA[:], triu[:])
                            psNi = ps.tile([128, HP], F32, name="psu", bufs=2,
                                           padded_shape=[128, 512])
                            nc.tensor.matmul(
                                psNi[:],
                                am[:, sub * 128:(sub + 1) * 128],
                                v_tm[c][:, h * HP:(h + 1) * HP],
                                start=True, stop=True)
                            nc.scalar.copy(ni[:, sub * HP:(sub + 1) * HP],
                                           psNi[:])
                        ni_sb[c][hp] = ni

                # ---- gather slots, combine prefix state P ------------------
                if DEBUG != "noremote":
                    nc.sync.dma_start(
                        slots.rearrange("p (j e) -> p j e", j=N_CORES),
                        cc_out.rearrange("j p e -> p j e"))

                PM = dpool.tile([128, W], F32, name="PM")
                PPc = [dpool.tile([128, W], BF16, name=f"PPc{c}")
                       for c in range(NCH)]
                if DEBUG == "noremote" or os.environ.get("LMA_NOGATHER") == "1":
                    nc.vector.tensor_copy(PM[:], mz[:])
                else:
                    nc.vector.scalar_tensor_tensor(
                        PM[:], slots[:, 0:W], pmask[:, 0:1], mz[:],
                        op0=Alu.mult, op1=Alu.add)
                    for k in range(1, N_CORES):
                        nc.vector.scalar_tensor_tensor(
                            PM[:], slots[:, k * W:(k + 1) * W],
                            pmask[:, k:k + 1], PM[:],
                            op0=Alu.mult, op1=Alu.add)
                nc.vector.tensor_copy(PPc[0][:], PM[:])
                for c in range(1, NCH):
                    nc.vector.tensor_add(PPc[c][:], PM[:], L_sb[c][:])

                # ---- numerators, divide, transpose -------------------------
                attnT = [dpool.tile([128, S_BLK], BF16, name=f"attnT{hp}")
                         for hp in range(NHP)]
                for c in range(NCH):
                    cs = slice(c * 128, (c + 1) * 128)
                    for hp in range(NHP):
                        ap_ = tpool.tile([128, 128], BF16, name="attnp")
                        for sub in range(2):
                            hb = slice(sub * 64, (sub + 1) * 64)
                            psN = ps.tile([128, HP], F32, name="psu", bufs=2,
                                          padded_shape=[128, 512])
                            nc.tensor.matmul(
                                psN[:], sqT[hp][hb, cs],
                                PPc[c][hb, hp * HP:(hp + 1) * HP],
                                start=True, stop=True)
                            nsl = slice(sub * HP, (sub + 1) * HP)
                            num = tpool.tile([128, HP], F32, name="numf",
                                             bufs=3)
                            nc.vector.tensor_add(num[:], psN[:],
                                                 ni_sb[c][hp][:, nsl])
                            den = spool.tile([128, 1], F32, name="den")
                            nc.vector.tensor_scalar_add(
                                den[:], num[:, DH:DH + 1], EPS)
                            rec = spool.tile([128, 1], F32, name="rec")
                            nc.vector.reciprocal(rec[:], den[:])
                            nc.vector.tensor_scalar_mul(
                                ap_[:, sub * DH:(sub + 1) * DH],
                                num[:, 0:DH], rec[:])
                        psT = ps.tile([128, 128], BF16, name="pstr", bufs=2)
                        nc.tensor.transpose(psT[:], ap_[:], ident[:])
                        nc.vector.tensor_copy(attnT[hp][:, cs], psT[:])

                # ---- output projection -------------------------------------
                for st in range(NCH):
                    ss = slice(st * 128, (st + 1) * 128)
                    psO = ps.tile([128, D], F32, name="psbig", bufs=2)
                    for hp in range(NHP):
                        nc.tensor.matmul(psO[:], attnT[hp][:, ss], wo_t[hp][:],
                                         start=(hp == 0), stop=(hp == NHP - 1))
                    y_sb = tpool.tile([128, D], F32, name="ysb", bufs=2)
                    nc.vector.tensor_copy(y_sb[:], psO[:])
                    nc.sync.dma_start(y_d[ss, :], y_sb[:])

    nc.compile()
    return nc


def _get_nc():
    if "nc" not in _CACHE:
        _CACHE["nc"] = _build()
    return _CACHE["nc"]


def _make_in_maps(hidden_states, Wq, bq, Wk, bk, Wv, bv, Wo, M_mem, z_mem):
    hs = np.asarray(hidden_states, np.float32).reshape(S, D)
    Wq = np.ascontiguousarray(np.asarray(Wq, np.float32))
    Wk = np.ascontiguousarray(np.asarray(Wk, np.float32))
    Wv = np.ascontiguousarray(np.asarray(Wv, np.float32))
    Wo = np.ascontiguousarray(np.asarray(Wo, np.float32))
    bq = np.asarray(bq, np.float32)
    bk = np.asarray(bk, np.float32)
    bv = np.asarray(bv, np.float32)
    M_mem = np.asarray(M_mem, np.float32)
    z_mem = np.asarray(z_mem, np.float32)

    mz = np.zeros((128, W), np.float32)
    for h in range(H):
        pr, col = (h % 2) * 64, (h // 2) * HP
        mz[pr:pr + 64, col:col + DH] = M_mem[h]
        mz[pr:pr + 64, col + DH] = z_mem[h]

    in_maps = []
    for c in range(N_CORES):
        aux = np.zeros((128, AUXW), np.float32)
        aux[:, 0:4] = bq.reshape(NHP, 128).T
        aux[:, 4:8] = bk.reshape(NHP, 128).T
        aux[:, 8:12] = bv.reshape(NHP, 128).T
        aux[:, 12:12 + W] = mz
        aux[:, 12 + W:12 + W + c] = 1.0
        sidx = np.full(128, c, dtype=np.uint32)
        in_maps.append({
            "hs": np.ascontiguousarray(hs[c * S_BLK:(c + 1) * S_BLK]),
            "wq": Wq, "wk": Wk, "wv": Wv, "wo": Wo,
            "aux": aux,
            "sidx": sidx.reshape(128, 1),
        })
    return in_maps


def kernel(hidden_states, Wq, bq, Wk, bk, Wv, bv, Wo, M_mem, z_mem):
    nc = _get_nc()
    in_maps = _make_in_maps(hidden_states, Wq, bq, Wk, bk, Wv, bv, Wo,
                            M_mem, z_mem)
    res = run_bass_kernel_spmd(nc, in_maps, list(range(N_CORES)))
    out = np.concatenate([res.results[c]["y"] for c in range(N_CORES)], axis=0)
    return out.reshape(1, S, D)


# revision 44
# speedup vs baseline: 1.2549x; 1.2549x over previous
"""Trainium2 Bass kernel for LinearMemoryAttention (B=1, S=4096, D=512, H=8, Dh=64).

v3: sequence-parallel over 8 cores (512 tokens each), all heads local.
- bf16 matmul operands throughout (fp32 PSUM accumulation).
- Projections computed feature-major so biases fuse into activations.
- Cross-core causal state exchanged through shared-HBM scratchpad: each
  core scatters its block-sum into its rank's slot (indirect DMA, slot
  index supplied as a per-core input), announces completion with a
  remote semaphore broadcast (SWDGE, no ncfw collective), then gathers
  all 8 slots with one DMA. A 1-byte prelude kernel barrier provides
  entry sync across invocations.

Self-contained: hardcodes all shapes; builds/compiles the Bass program once.
"""

import os

import numpy as np

import concourse.bass as bass
import concourse.bacc as bacc
import concourse.mybir as mybir
import concourse.tile as tile
from concourse.bass_utils import run_bass_kernel_spmd

F32 = mybir.dt.float32
BF16 = mybir.dt.bfloat16
U32 = mybir.dt.uint32
N_CORES = 8
S = 4096
D = 512
H = 8
DH = 64
HP = 66  # head width incl. denominator column (+1 pad)
S_BLK = S // N_CORES  # 512 rows per core
NCH = S_BLK // 128  # 4 chunks of 128
NHP = H // 2  # 4 head pairs
EPS = 1e-6
W = NHP * HP  # 264
AUXW = 4 + 4 + 4 + W + N_CORES  # bqt | bkt | bvt | mz | pmask

_CACHE = {}
DEBUG = os.environ.get("LMA_DEBUG", "")  # "" or "noremote"


def _build():
    Alu = mybir.AluOpType
    Act = mybir.ActivationFunctionType
    nc = bacc.Bacc("TRN2", target_bir_lowering=False, debug=False,
                   num_devices=N_CORES)

    hs_d = nc.dram_tensor("hs", [S_BLK, D], F32, kind="ExternalInput").ap()
    wq_d = nc.dram_tensor("wq", [D, D], F32, kind="ExternalInput").ap()
    wk_d = nc.dram_tensor("wk", [D, D], F32, kind="ExternalInput").ap()
    wv_d = nc.dram_tensor("wv", [D, D], F32, kind="ExternalInput").ap()
    wo_d = nc.dram_tensor("wo", [D, D], F32, kind="ExternalInput").ap()
    aux_d = nc.dram_tensor("aux", [128, AUXW], F32, kind="ExternalInput").ap()
    sidx_d = nc.dram_tensor("sidx", [128, 1], U32, kind="ExternalInput").ap()
    y_d = nc.dram_tensor("y", [S_BLK, D], F32, kind="ExternalOutput").ap()

    rsem = nc.alloc_semaphore("lma_rsem")
    lsem = nc.alloc_semaphore("lma_lsem")
    dsem = nc.alloc_semaphore("lma_dsem")

    with tile.TileContext(nc) as tc:
        with (
            tc.tile_pool(name="const", bufs=1) as cpool,
            tc.tile_pool(name="wstage", bufs=1) as wspool,
            tc.tile_pool(name="wpool", bufs=1) as wpool,
            tc.tile_pool(name="data", bufs=1) as dpool,
            tc.tile_pool(name="tmp", bufs=3) as tpool,
            tc.tile_pool(name="small", bufs=4) as spool,
            tc.tile_pool(name="dram", bufs=1, space="DRAM") as drpool,
        ):
            # ---- input DMAs (one issue per tensor, sync queue = idle) ------
            hs_t = dpool.tile([128, NCH * D], F32, name="hsall")
            nc.sync.dma_start(
                hs_t.rearrange("p (c d) -> p c d", c=NCH),
                hs_d.rearrange("(c p) d -> p c d", p=128))
            aux = cpool.tile([128, AUXW], F32)
            nc.sync.dma_start(aux[:], aux_d[:])
            sidx = cpool.tile([128, 1], U32)
            nc.sync.dma_start(sidx[:], sidx_d[:])
            wk_s = wspool.tile([128, 4 * D], F32, name="wks")
            nc.sync.dma_start(
                wk_s.rearrange("p (c d) -> p c d", c=4),
                wk_d.rearrange("(c p) d -> p c d", p=128))
            wv_s = wspool.tile([128, 4 * D], F32, name="wvs")
            nc.sync.dma_start(
                wv_s.rearrange("p (c d) -> p c d", c=4),
                wv_d.rearrange("(c p) d -> p c d", p=128))
            wq_s = wspool.tile([128, 4 * D], F32, name="wqs")
            nc.sync.dma_start(
                wq_s.rearrange("p (c d) -> p c d", c=4),
                wq_d.rearrange("(c p) d -> p c d", p=128))
            wo_s = wspool.tile([128, 4 * D], F32, name="wos")
            nc.sync.dma_start(
                wo_s.rearrange("p (c d) -> p c d", c=4),
                wo_d.rearrange("(c p) d -> p c d", p=128))

            bqt = aux[:, 0:4]
            bkt = aux[:, 4:8]
            bvt = aux[:, 8:12]
            mz = aux[:, 12:12 + W]
            pmask = aux[:, 12 + W:12 + W + N_CORES]

            # ---- constants -------------------------------------------------
            ones128 = cpool.tile([128, 128], BF16)
            nc.gpsimd.memset(ones128[:], 1.0)
            ident = cpool.tile([128, 128], BF16)
            nc.gpsimd.affine_select(ident[:], ones128[:], pattern=[[1, 128]],
                                    compare_op=Alu.is_equal, fill=0.0, base=0,
                                    channel_multiplier=-1)
            triu = cpool.tile([128, 128], BF16)
            nc.gpsimd.affine_select(triu[:], ones128[:], pattern=[[1, 128]],
                                    compare_op=Alu.is_ge, fill=0.0, base=0,
                                    channel_multiplier=-1)

            # ---- cast hs + weights to bf16 --------------------------------
            hs_b = dpool.tile([128, NCH * D], BF16, name="hsb")
            for c in range(NCH):
                nc.vector.tensor_copy(hs_b[:, c * D:(c + 1) * D],
                                      hs_t[:, c * D:(c + 1) * D])
            wk_t = [wpool.tile([128, D], BF16, name=f"wk{i}") for i in range(4)]
            wv_t = [wpool.tile([128, D], BF16, name=f"wv{i}") for i in range(4)]
            wq_t = [wpool.tile([128, D], BF16, name=f"wq{i}") for i in range(4)]
            wo_t = [wpool.tile([128, D], BF16, name=f"wo{i}") for i in range(4)]
            for i in range(4):
                sl = slice(i * D, (i + 1) * D)
                nc.vector.tensor_copy(wk_t[i][:], wk_s[:, sl])
                nc.vector.tensor_copy(wv_t[i][:], wv_s[:, sl])
                nc.scalar.copy(wq_t[i][:], wq_s[:, sl])
                nc.scalar.copy(wo_t[i][:], wo_s[:, sl])

            # ---- cross-core exchange buffers ------------------------------
            ball = dpool.tile([128, W], BF16, name="ball")
            slots = dpool.tile([128, N_CORES * W], BF16, name="slots")


            hsT = [dpool.tile([128, S_BLK], BF16, name=f"hsT{i}")
                   for i in range(4)]
            skT = [dpool.tile([128, S_BLK], BF16, name=f"skT{hp}")
                   for hp in range(NHP)]
            sqT = [dpool.tile([128, S_BLK], BF16, name=f"sqT{hp}")
                   for hp in range(NHP)]
            vT = [dpool.tile([128, S_BLK], BF16, name=f"vT{hp}")
                  for hp in range(NHP)]
            sk_tm = [dpool.tile([128, H * DH], BF16, name=f"sk{c}")
                     for c in range(NCH)]
            v_tm = [dpool.tile([128, H * HP], BF16, name=f"v{c}")
                    for c in range(NCH)]
            for c in range(NCH):
                v3 = v_tm[c].rearrange("p (h e) -> p h e", e=HP)
                nc.gpsimd.memset(v3[:, :, DH:HP], 1.0)
            L_sb = [None] + [dpool.tile([128, W], F32, name=f"L{c}")
                             for c in range(1, NCH)]

            with tc.tile_pool(name="ps", bufs=1, space="PSUM") as ps:
                # ---- hs transposes -----------------------------------------
                for dt in range(4):
                    for st in range(4):
                        pst = ps.tile([128, 128], BF16, name="pstr", bufs=2)
                        nc.tensor.transpose(
                            pst[:],
                            hs_b[:, st * D + dt * 128: st * D + (dt + 1) * 128],
                            ident[:])
                        nc.vector.tensor_copy(
                            hsT[dt][:, st * 128:(st + 1) * 128], pst[:])

                # ---- k, v projections (feature-major), elu(k)+1 ------------
                for hp in range(NHP):
                    fs = slice(hp * 128, (hp + 1) * 128)
                    psk = ps.tile([128, S_BLK], F32, name="psbig", bufs=2)
                    for dt in range(4):
                        nc.tensor.matmul(psk[:], wk_t[dt][:, fs], hsT[dt][:],
                                         start=(dt == 0), stop=(dt == 3))
                    e_t = tpool.tile([128, S_BLK], BF16, name="elu_e")
                    r_t = tpool.tile([128, S_BLK], BF16, name="elu_r")
                    nc.scalar.activation(e_t[:], psk[:], Act.Exp,
                                         bias=bkt[:, hp:hp + 1])
                    nc.vector.tensor_scalar(r_t[:], psk[:], bkt[:, hp:hp + 1],
                                            0.0, op0=Alu.add, op1=Alu.max)
                    nc.vector.scalar_tensor_tensor(
                        skT[hp][:], e_t[:], 1.0, r_t[:],
                        op0=Alu.min, op1=Alu.add)

                    psv = ps.tile([128, S_BLK], F32, name="psbig", bufs=2)
                    for dt in range(4):
                        nc.tensor.matmul(psv[:], wv_t[dt][:, fs], hsT[dt][:],
                                         start=(dt == 0), stop=(dt == 3))
                    nc.scalar.activation(vT[hp][:], psv[:], Act.Identity,
                                         bias=bvt[:, hp:hp + 1])

                # ---- transpose sk, v to token-major ------------------------
                for c in range(NCH):
                    cs = slice(c * 128, (c + 1) * 128)
                    for hp in range(NHP):
                        pst = ps.tile([128, 128], BF16, name="pstr", bufs=2)
                        nc.tensor.transpose(pst[:], skT[hp][:, cs], ident[:])
                        nc.vector.tensor_copy(
                            sk_tm[c][:, hp * 128:(hp + 1) * 128], pst[:])
                        pst2 = ps.tile([128, 128], BF16, name="pstr", bufs=2)
                        nc.tensor.transpose(pst2[:], vT[hp][:, cs], ident[:])
                        v3 = v_tm[c].rearrange("p (h e) -> p h e", e=HP)
                        nc.scalar.copy(
                            v3[:, 2 * hp:2 * hp + 2, 0:DH],
                            pst2.rearrange("p (h e) -> p h e", e=DH))

                # ---- block total B: accumulate all chunks on PE ------------
                for hp in range(NHP):
                    psB = ps.tile([128, HP], F32, name="psa", bufs=2,
                                  padded_shape=[128, 512])
                    for st in range(NCH):
                        for sub in range(2):
                            h = 2 * hp + sub
                            nc.tensor.matmul(
                                psB[sub * 64:(sub + 1) * 64, :],
                                sk_tm[st][:, h * DH:(h + 1) * DH],
                                v_tm[st][:, h * HP:(h + 1) * HP],
                                start=(st == 0), stop=(st == NCH - 1),
                                tile_position=(0, 64 * sub))
                    nc.scalar.copy(ball[:, hp * HP:(hp + 1) * HP], psB[:])

                # ---- U outer products -> local chunk prefixes --------------
                for st in range(NCH - 1):
                    for hp in range(NHP):
                        psU = ps.tile([128, HP], F32, name="psu", bufs=2,
                                      padded_shape=[128, 512])
                        for sub in range(2):
                            h = 2 * hp + sub
                            nc.tensor.matmul(
                                psU[sub * 64:(sub + 1) * 64, :],
                                sk_tm[st][:, h * DH:(h + 1) * DH],
                                v_tm[st][:, h * HP:(h + 1) * HP],
                                start=True, stop=True,
                                tile_position=(0, 64 * sub))
                        dsl = slice(hp * HP, (hp + 1) * HP)
                        nc.scalar.copy(L_sb[st + 1][:, dsl], psU[:])
                for st in range(2, NCH):
                    nc.gpsimd.tensor_add(L_sb[st][:], L_sb[st][:],
                                         L_sb[st - 1][:])

                # ---- exchange: AllGather block totals (bf16 payload) -------
                if DEBUG != "noremote":
                    cc_in = drpool.tile([128, W], BF16, name="ccin")
                    cc_out = drpool.tile([N_CORES, 128, W], BF16,
                                         addr_space="Shared", name="ccout")
                    nc.sync.dma_start(cc_in[:], ball[:])
                    nc.gpsimd.collective_compute(
                        "AllGather", Alu.bypass,
                        replica_groups=[list(range(N_CORES))],
                        ins=[cc_in[:]], outs=[cc_out[:]])

                # ---- q projections (overlap the exchange) ------------------
                for hp in range(NHP):
                    fs = slice(hp * 128, (hp + 1) * 128)
                    psq = ps.tile([128, S_BLK], F32, name="psbig", bufs=2)
                    for dt in range(4):
                        nc.tensor.matmul(psq[:], wq_t[dt][:, fs], hsT[dt][:],
                                         start=(dt == 0), stop=(dt == 3))
                    e_t = tpool.tile([128, S_BLK], BF16, name="elu_e")
                    r_t = tpool.tile([128, S_BLK], BF16, name="elu_r")
                    nc.scalar.activation(e_t[:], psq[:], Act.Exp,
                                         bias=bqt[:, hp:hp + 1])
                    nc.vector.tensor_scalar(r_t[:], psq[:], bqt[:, hp:hp + 1],
                                            0.0, op0=Alu.add, op1=Alu.max)
                    nc.vector.scalar_tensor_tensor(
                        sqT[hp][:], e_t[:], 1.0, r_t[:],
                        op0=Alu.min, op1=Alu.add)

                # ---- masked intra-chunk scores -----------------------------
                am_sb = [[None] * NHP for _ in range(NCH)]
                for c in range(NCH):
                    cs = slice(c * 128, (c + 1) * 128)
                    for hp in range(NHP):
                        am = dpool.tile([128, 256], BF16, name=f"am{c}_{hp}")
                        for sub in range(2):
                            hb = slice(sub * 64, (sub + 1) * 64)
                            psA = ps.tile([128, 128], F32, name="psa", bufs=2)
                            nc.tensor.matmul(psA[:], skT[hp][hb, cs],
                                             sqT[hp][hb, cs],
                                             start=True, stop=True)
                            nc.vector.tensor_mul(
                                am[:, sub * 128:(sub + 1) * 128],
                                psA[:], triu[:])
                        am_sb[c][hp] = am

                # ---- gather slots, combine prefix state P ------------------
                if DEBUG != "noremote":
                    nc.sync.dma_start(
                        slots.rearrange("p (j e) -> p j e", j=N_CORES),
                        cc_out.rearrange("j p e -> p j e"))

                PM = dpool.tile([128, W], F32, name="PM")
                PPc = [dpool.tile([128, W], BF16, name=f"PPc{c}")
                       for c in range(NCH)]
                if DEBUG == "noremote" or os.environ.get("LMA_NOGATHER") == "1":
                    nc.vector.tensor_copy(PM[:], mz[:])
                else:
                    nc.vector.scalar_tensor_tensor(
                        PM[:], slots[:, 0:W], pmask[:, 0:1], mz[:],
                        op0=Alu.mult, op1=Alu.add)
                    for k in range(1, N_CORES):
                        nc.vector.scalar_tensor_tensor(
                            PM[:], slots[:, k * W:(k + 1) * W],
                            pmask[:, k:k + 1], PM[:],
                            op0=Alu.mult, op1=Alu.add)
                nc.vector.tensor_copy(PPc[0][:], PM[:])
                for c in range(1, NCH):
                    nc.vector.tensor_add(PPc[c][:], PM[:], L_sb[c][:])

                # ---- numerators, divide, transpose -------------------------
                attnT = [dpool.tile([128, S_BLK], BF16, name=f"attnT{hp}")
                         for hp in range(NHP)]
                for c in range(NCH):
                    cs = slice(c * 128, (c + 1) * 128)
                    for hp in range(NHP):
                        ap_ = tpool.tile([128, 128], BF16, name="attnp")
                        for sub in range(2):
                            h = 2 * hp + sub
                            hb = slice(sub * 64, (sub + 1) * 64)
                            psN = ps.tile([128, HP], F32, name="psu", bufs=2,
                                          padded_shape=[128, 512])
                            nc.tensor.matmul(
                                psN[:],
                                am_sb[c][hp][:, sub * 128:(sub + 1) * 128],
                                v_tm[c][:, h * HP:(h + 1) * HP],
                                start=True, stop=False)
                            nc.tensor.matmul(
                                psN[:], sqT[hp][hb, cs],
                                PPc[c][hb, hp * HP:(hp + 1) * HP],
                                start=False, stop=True)
                            den = spool.tile([128, 1], F32, name="den")
                            nc.vector.tensor_scalar_add(
                                den[:], psN[:, DH:DH + 1], EPS)
                            rec = spool.tile([128, 1], F32, name="rec")
                            nc.vector.reciprocal(rec[:], den[:])
                            nc.vector.tensor_scalar_mul(
                                ap_[:, sub * DH:(sub + 1) * DH],
                                psN[:, 0:DH], rec[:])
                        psT = ps.tile([128, 128], BF16, name="pstr", bufs=2)
                        nc.tensor.transpose(psT[:], ap_[:], ident[:])
                        nc.vector.tensor_copy(attnT[hp][:, cs], psT[:])

                # ---- output projection -------------------------------------
                for st in range(NCH):
                    ss = slice(st * 128, (st + 1) * 128)
                    psO = ps.tile([128, D], F32, name="psbig", bufs=2)
                    for hp in range(NHP):
                        nc.tensor.matmul(psO[:], attnT[hp][:, ss], wo_t[hp][:],
                                         start=(hp == 0), stop=(hp == NHP - 1))
                    y_sb = tpool.tile([128, D], F32, name="ysb", bufs=2)
                    nc.vector.tensor_copy(y_sb[:], psO[:])
                    nc.sync.dma_start(y_d[ss, :], y_sb[:])

    nc.compile()
    return nc


def _get_nc():
    if "nc" not in _CACHE:
        _CACHE["nc"] = _build()
    return _CACHE["nc"]


def _make_in_maps(hidden_states, Wq, bq, Wk, bk, Wv, bv, Wo, M_mem, z_mem):
    hs = np.asarray(hidden_states, np.float32).reshape(S, D)
    Wq = np.ascontiguousarray(np.asarray(Wq, np.float32))
    Wk = np.ascontiguousarray(np.asarray(Wk, np.float32))
    Wv = np.ascontiguousarray(np.asarray(Wv, np.float32))
    Wo = np.ascontiguousarray(np.asarray(Wo, np.float32))
    bq = np.asarray(bq, np.float32)
    bk = np.asarray(bk, np.float32)
    bv = np.asarray(bv, np.float32)
    M_mem = np.asarray(M_mem, np.float32)
    z_mem = np.asarray(z_mem, np.float32)

    mz = np.zeros((128, W), np.float32)
    for h in range(H):
        pr, col = (h % 2) * 64, (h // 2) * HP
        mz[pr:pr + 64, col:col + DH] = M_mem[h]
        mz[pr:pr + 64, col + DH] = z_mem[h]

    in_maps = []
    for c in range(N_CORES):
        aux = np.zeros((128, AUXW), np.float32)
        aux[:, 0:4] = bq.reshape(NHP, 128).T
        aux[:, 4:8] = bk.reshape(NHP, 128).T
        aux[:, 8:12] = bv.reshape(NHP, 128).T
        aux[:, 12:12 + W] = mz
        aux[:, 12 + W:12 + W + c] = 1.0
        sidx = np.full(128, c, dtype=np.uint32)
        in_maps.append({
            "hs": np.ascontiguousarray(hs[c * S_BLK:(c + 1) * S_BLK]),
            "wq": Wq, "wk": Wk, "wv": Wv, "wo": Wo,
            "aux": aux,
            "sidx": sidx.reshape(128, 1),
        })
    return in_maps


def kernel(hidden_states, Wq, bq, Wk, bk, Wv, bv, Wo, M_mem, z_mem):
    nc = _get_nc()
    in_maps = _make_in_maps(hidden_states, Wq, bq, Wk, bk, Wv, bv, Wo,
                            M_mem, z_mem)
    res = run_bass_kernel_spmd(nc, in_maps, list(range(N_CORES)))
    out = np.concatenate([res.results[c]["y"] for c in range(N_CORES)], axis=0)
    return out.reshape(1, S, D)
